# revision 5
# baseline (speedup 1.0000x reference)
"""Trainium2 Bass kernel for the AggregateLayer pooling problem.

reference semantics (per batch b):
    dot_w[j] = <pref[b,j,:], c[b,0,:]>                      (j = 0..63)
    t_w[j]   = 1 / |t_pref[b,0,j] - t_c[b,0]|
    w        = softmax(dot_w + t_w)                          (over j)
    u[b,0,:] = sum_j w[j] * pref[b,j,:]

Strategy: pure data parallel over 8 NeuronCores (1024 batches each),
batches in groups of GROUP=128 (64 two-batch tiles of 128 flattened
(batch, j) rows x 128 D cols).

Engine budget (measured): pref stream on the gpsimd SWDGE ring runs at
~712 GB/s (47 us); PE tiny matmuls (dot + weighted-sum, N=2) pipeline
their weight loads and cost ~36 ns each (37 us for all 1024); PE
transposes cost ~95 ns each; the XBAR dma-transpose (sync HWDGE) moves
a 128x128 fp16 tile in ~172 ns but is a single shared resource --
concurrent XPOSEs from two HWDGE queues corrupt each other's output
(measured), so ONLY the sync queue ever issues them, and XPOSE
destinations must be contiguous per partition.

The per-group transpose work (needed only to compute the dots) is
split between the XBAR (chunks 0..XCH-1) and PE+PSUM-copy (chunks
XCH..7), balancing PE ~57 us vs XBAR ~55 us. The pipeline runs the
transposes of group g+1 (both paths) during group g's dots/softmax/
weighted-sum, pref loads two groups ahead, and the weighted sum one
group behind, so no engine queue waits mid-stream. u is stored
transposed [D, BPC] straight from the weighted-sum PSUM layout; the
host transposes it back.
"""

import numpy as np
from contextlib import ExitStack

import concourse.bass as bass
import concourse.tile as tile
from concourse import mybir
from concourse.masks import make_identity
from concourse.bass_utils import run_bass_kernel_spmd
import concourse.bass2jax as _b2j


def _split_multiwait(bir: dict) -> int:
    """Walrus in this container rejects >1 sync-wait per instruction.

    Hoist excess waits onto NoOps inserted just before the instruction on
    the same engine (program order within the engine stream preserves the
    wait semantics exactly).
    """
    n = 0
    for fn in bir["functions"]:
        for blk in fn["blocks"]:
            out = []
            for inst in blk["instructions"]:
                si = inst.get("sync_info")
                waits = si.get("on_wait") if si else None
                if waits and len(waits) > 1:
                    for w in waits[:-1]:
                        out.append(
                            {
                                "opcode": "NoOp",
                                "engine": inst["engine"],
                                "name": f"{inst['name']}-xw{n}",
                                "ins": [],
                                "outs": [],
                                "sync_info": {"on_update": [], "on_wait": [w]},
                            }
                        )
                        n += 1
                    si["on_wait"] = [waits[-1]]
                out.append(inst)
            blk["instructions"] = out
    return n


_orig_compile_bir_kernel = _b2j.compile_bir_kernel


def _legalizing_compile_bir_kernel(ant_bir_str, *args, **kwargs):
    import orjson

    bir = orjson.loads(ant_bir_str)
    _split_multiwait(bir)
    return _orig_compile_bir_kernel(orjson.dumps(bir), *args, **kwargs)


_b2j.compile_bir_kernel = _legalizing_compile_bir_kernel

F32 = mybir.dt.float32
F16 = mybir.dt.float16
Alu = mybir.AluOpType
Act = mybir.ActivationFunctionType
Axis = mybir.AxisListType

B, N, D = 8192, 64, 128
NCORES = 8
BPC = B // NCORES          # 1024 batches per core
GROUP = 128                # batches per group
NGROUPS = BPC // GROUP     # 8
NTILES = GROUP // 2        # 64 two-batch tiles per group
NPAIR = GROUP // 2         # 64 batch-pairs per group (softmax partitions)
CH = 8                     # tiles per transpose/copy/dot chunk
NCH = NTILES // CH         # 8 chunks per group
XCH = 5                    # chunks transposed via XBAR (rest via PE)
HT = 16                    # tiles per pref DMA chunk


class _St:
    """Per-group pipeline state carried between build phases."""

    def __init__(self, g):
        self.g = g
        self.p16 = None
        self.pts = None
        self.tw = None
        self.w = None
        self.nmx = None
        self.wn16 = None
        self.wmat16 = None


class _Ctx:
    def __init__(self, tc, pools, consts, aps):
        self.tc = tc
        self.nc = tc.nc
        (self.p_p16, self.p_pts, self.p_small, self.ps_pt, self.ps_mm,
         self.ps_small) = pools
        self.ident16, self.ident32 = consts
        (self.pref_rows, self.u_all, self.ct16a, self.tpa, self.tca,
         self.cg16, self.wmat_ring, self.p16s, self.ptss) = aps


def _emit_pref_loads(cx, g, skip=0):
    """pref chunk DMAs for group g on the gpsimd SWDGE (fp32->fp16)."""
    nc = cx.nc
    r0 = g * GROUP * N
    for h0 in range(skip, NTILES, HT):
        rh = r0 + h0 * 128
        nc.gpsimd.dma_start(
            out=cx.p16s[g][:, h0 : h0 + HT, :],
            in_=cx.pref_rows[rh : rh + HT * 128, :].rearrange(
                "(t p) d -> p t d", p=128
            ),
        )


def _emit_xbar(cx, g):
    """XBAR transposes for chunks 0..XCH-1 of group g (sync queue ONLY).

    Destination slices are contiguous per partition (full-width innermost
    slices of the pts tile) -- required for correct XPOSE output. One big
    XPOSE per group: fewer HWDGE instructions entangle less with the
    SWDGE vector-clock waits (each extra XPOSE adds spurious cross-ring
    ordering edges on later gpsimd loads).
    """
    nc = cx.nc
    nt = XCH * CH
    nc.sync.dma_start_transpose(
        out=cx.ptss[g][:, 0:nt, :],
        in_=cx.p16s[g][:, 0:nt, :].rearrange("p t d -> p (t d)"),
    )


def _emit_pe_T(cx, g):
    """PE transposes for chunks XCH..NCH-1 of group g + PSUM->SBUF copies."""
    nc = cx.nc
    for k in range(XCH, NCH):
        t0 = k * CH
        pt_ps = cx.ps_pt.tile(
            [128, CH, 128], F16, tag="pt_ps", name=f"ptps{g}_{k}"
        )
        for i in range(CH):
            nc.tensor.transpose(
                out=pt_ps[:, i, :],
                in_=cx.p16s[g][:, t0 + i, :],
                identity=cx.ident16[:],
            )
        if k == XCH:
            nc.vector.tensor_copy(
                out=cx.ptss[g][:, t0 : t0 + CH, :], in_=pt_ps[:]
            )
        else:
            nc.scalar.copy(out=cx.ptss[g][:, t0 : t0 + CH, :], in_=pt_ps[:])


def _phase_dots(cx, g):
    """tw + dot matmuls from pts + extraction + add/max + softmax tail."""
    nc = cx.nc
    st = _St(g)
    st.p16 = cx.p16s[g]
    st.pts = cx.ptss[g]

    st.tw = cx.p_small.tile([NPAIR, 2, N], F32, tag="tw", name=f"tw{g}")
    for s in range(2):
        nc.vector.tensor_scalar_sub(
            out=st.tw[:, s, :],
            in0=cx.tpa[:, g, s, :],
            scalar1=cx.tca[:, g, s : s + 1],
        )
    nc.scalar.activation(out=st.tw[:], in_=st.tw[:], func=Act.Abs)
    nc.vector.reciprocal(out=st.tw[:], in_=st.tw[:])

    ps_dots = cx.ps_mm.tile(
        [128, NTILES, 2], F32, tag="mm_ps", name=f"dots{g}"
    )
    for k in range(NCH):
        t0 = k * CH
        for i in range(CH):
            t = t0 + i
            nc.tensor.matmul(
                out=ps_dots[:, t, :],
                lhsT=st.pts[:, t, :],
                rhs=cx.ct16a[:, g, 2 * t : 2 * t + 2],
                start=(i == 0),
                stop=(i == CH - 1),
            )

    # valid dots sit at [row, parity=row//64]: extract the two halves
    dotw = cx.p_small.tile([128, NTILES], F32, tag="dotw", name=f"dotw{g}")
    nc.scalar.copy(out=dotw[0:64, :], in_=ps_dots[0:64, :, 0])
    nc.scalar.copy(out=dotw[64:128, :], in_=ps_dots[64:128, :, 1])

    # transpose [128(row), nt] -> [nt, 128(row)] => pair-major dots
    dr_ps = cx.ps_small.tile([NPAIR, 128], F32, tag="sm_ps", name=f"dr{g}")
    nc.tensor.transpose(out=dr_ps[:], in_=dotw[:], identity=cx.ident32[:])

    st.w = cx.p_small.tile([NPAIR, 2, N], F32, tag="w", name=f"w{g}")
    nc.vector.tensor_add(
        out=st.w[:],
        in0=dr_ps[:].rearrange("t (two n) -> t two n", two=2),
        in1=st.tw[:],
    )
    st.nmx = cx.p_small.tile([NPAIR, 2], F32, tag="nmx", name=f"nmx{g}")
    nc.vector.tensor_reduce(
        out=st.nmx[:], in_=st.w[:], axis=Axis.X, op=Alu.max, negate=True
    )
    _phase_b1(cx, st)
    return st


def _phase_b1(cx, st):
    """Softmax tail: exp + sum + reciprocal + normalize (no PE)."""
    nc = cx.nc
    g = st.g
    e = cx.p_small.tile([NPAIR, 2, N], F32, tag="e", name=f"e{g}")
    for s in range(2):
        nc.scalar.activation(
            out=e[:, s, :],
            in_=st.w[:, s, :],
            func=Act.Exp,
            bias=st.nmx[:, s : s + 1],
            scale=1.0,
        )
    z = cx.p_small.tile([NPAIR, 2], F32, tag="z", name=f"z{g}")
    nc.vector.reduce_sum(out=z[:], in_=e[:], axis=Axis.X)
    rz = cx.p_small.tile([NPAIR, 2], F32, tag="rz", name=f"rz{g}")
    nc.vector.reciprocal(out=rz[:], in_=z[:])
    st.wn16 = cx.p_small.tile([NPAIR, 2, N], F16, tag="wn16", name=f"wn{g}")
    for s in range(2):
        nc.vector.tensor_scalar_mul(
            out=st.wn16[:, s, :], in0=e[:, s, :], scalar1=rz[:, s : s + 1]
        )


def _phase_b2(cx, st):
    """W_MAT build: PE transpose of wn16 + block scatter."""
    nc = cx.nc
    g = st.g
    wc_ps = cx.ps_small.tile([128, NTILES], F16, tag="sm_ps", name=f"wc{g}")
    nc.tensor.transpose(
        out=wc_ps[:],
        in_=st.wn16[:].rearrange("t two n -> t (two n)"),
        identity=cx.ident16[0:NPAIR, 0:NPAIR],
    )
    wcol = cx.p_small.tile([128, NTILES], F16, tag="wcol", name=f"wcol{g}")
    nc.vector.tensor_copy(out=wcol[:], in_=wc_ps[:])
    # persistent pre-zeroed ring: only the data halves are ever written,
    # the zero halves survive across generations
    st.wmat16 = cx.wmat_ring[g % len(cx.wmat_ring)]
    nc.vector.tensor_copy(out=st.wmat16[0:64, :, 0], in_=wcol[0:64, :])
    nc.vector.tensor_copy(out=st.wmat16[64:128, :, 1], in_=wcol[64:128, :])


def _phase_c(cx, st):
    """Weighted-sum matmuls + u extraction (DVE) + store (gpsimd)."""
    nc = cx.nc
    g = st.g
    b0 = g * GROUP
    HB = NTILES // 2
    for h in range(2):
        ps_ut = cx.ps_mm.tile(
            [128, HB, 2], F32, tag="mm_ps", name=f"ut{g}_{h}"
        )
        for k in range(HB):
            t = h * HB + k
            nc.tensor.matmul(
                out=ps_ut[:, k, :],
                lhsT=st.p16[:, t, :],
                rhs=st.wmat16[:, t, :],
                start=(k == 0),
                stop=(k == HB - 1),
            )
        uts = cx.p_small.tile(
            [128, GROUP // 2], F32, tag="uts", name=f"uts{g}_{h}"
        )
        nc.vector.tensor_copy(
            out=uts[:], in_=ps_ut[:].rearrange("d t two -> d (t two)")
        )
        bh = b0 + h * (GROUP // 2)
        nc.gpsimd.dma_start(
            out=cx.u_all[:, bh : bh + GROUP // 2], in_=uts[:]
        )


def _emit_ct(cx, g):
    """PE transpose of group g's c half into ct16a (+ DVE copy)."""
    nc = cx.nc
    ct_ps = cx.ps_small.tile([128, 128], F16, tag="sm_ps", name=f"ct{g}")
    nc.tensor.transpose(
        out=ct_ps[:],
        in_=cx.cg16[:, g, :],
        identity=cx.ident16[:],
    )
    nc.vector.tensor_copy(out=cx.ct16a[:, g, :], in_=ct_ps[:])


def _build_nc():
    nc = bass.Bass()
    pref = nc.declare_dram_parameter("pref", [BPC, N, D], F32, isOutput=False)
    c = nc.declare_dram_parameter("c", [BPC, 1, D], F32, isOutput=False)
    t_pref = nc.declare_dram_parameter("t_pref", [BPC, 1, N], F32, isOutput=False)
    t_c = nc.declare_dram_parameter("t_c", [BPC, 1], F32, isOutput=False)
    # u stored transposed [D, BPC] (direct from the weighted-sum PSUM
    # layout); the host transposes it back.
    u = nc.declare_dram_parameter("u", [D, BPC], F32, isOutput=True)

    pref_rows = pref[:].rearrange("b n d -> (b n) d")
    c_all = c[:].rearrange("b one d -> (b one) d")
    tp_all = t_pref[:].rearrange("b one n -> (b one) n")
    tc_all = t_c[:]
    u_all = u[:]

    with ExitStack() as ctx:
        tc = ctx.enter_context(tile.TileContext(nc))
        p_const = ctx.enter_context(tc.tile_pool(name="const", bufs=1))
        p_pre = ctx.enter_context(tc.tile_pool(name="pre", bufs=1))
        # bufs=8: every group keeps its own p16 slot for the whole run.
        # No ring reuse -> the pref loads carry ZERO WAR edges, and since
        # they are all emitted before any XPOSE they cannot inherit the
        # scheduler's conservative cross-ring (SWDGE<->HWDGE) waits that
        # serialized the pipeline when loads trailed XPOSEs.
        p_p16 = ctx.enter_context(tc.tile_pool(name="p16", bufs=8))
        p_pts = ctx.enter_context(tc.tile_pool(name="pts", bufs=3))
        p_small = ctx.enter_context(tc.tile_pool(name="small", bufs=3))
        ps_pt = ctx.enter_context(tc.tile_pool(name="ps_pt", bufs=3, space="PSUM"))
        ps_mm = ctx.enter_context(tc.tile_pool(name="ps_mm", bufs=3, space="PSUM"))
        ps_small = ctx.enter_context(
            tc.tile_pool(name="ps_small", bufs=2, space="PSUM")
        )

        nb = NGROUPS * GROUP

        # persistent rings
        p16s = []
        ptss = []
        for _gi in range(NGROUPS):
            p16s.append(
                p_p16.tile([128, NTILES, D], F16, tag="p16", name=f"p16_{_gi}")
            )
            ptss.append(
                p_pts.tile([128, NTILES, D], F16, tag="pts", name=f"pts_{_gi}")
            )

        # gpsimd stream head: group 0's first pref chunk, then identity
        # builds (gpsimd-only affine_select), then c/t loads, then the
        # rest of pref.
        nc.gpsimd.dma_start(
            out=p16s[0][:, 0:HT, :],
            in_=pref_rows[0 : HT * 128, :].rearrange("(t p) d -> p t d", p=128),
        )
        ident16 = p_const.tile([128, 128], F16)
        make_identity(nc, ident16[:])

        c32a = p_pre.tile([128, NGROUPS, D], F32)
        nc.gpsimd.dma_start(
            out=c32a[:],
            in_=c_all[0:nb, :].rearrange("(g b) d -> b g d", b=128),
        )
        tpa = p_pre.tile([NPAIR, NGROUPS, 2, N], F32)
        nc.gpsimd.dma_start(
            out=tpa[:],
            in_=tp_all[0:nb, :].rearrange(
                "(g t two) n -> t g two n", t=NPAIR, two=2
            ),
        )
        tca = p_pre.tile([NPAIR, NGROUPS, 2], F32)
        nc.gpsimd.dma_start(
            out=tca[:],
            in_=tc_all[0:nb, :].rearrange(
                "(g t two) one -> t g (two one)", t=NPAIR, two=2
            ),
        )
        ident32 = p_const.tile([128, 128], F32)
        make_identity(nc, ident32[:])
        consts = (ident16, ident32)

        _emit_pref_loads_head = True
        # rest of group 0 + group 1 pref
        # (emitted below via cx; build cx first)
        cg16 = p_pre.tile([128, NGROUPS, D], F16)
        ct16a = p_pre.tile([128, NGROUPS, 128], F16)  # [D, group, batch]

        wmat_ring = []
        for _wi in range(3):
            wm = p_pre.tile([128, NTILES, 2], F16, name=f"wmatr{_wi}")
            nc.vector.memset(wm[:], 0.0)
            wmat_ring.append(wm)

        aps = (pref_rows, u_all, ct16a, tpa, tca, cg16, wmat_ring, p16s, ptss)
        cx = _Ctx(tc, (p_p16, p_pts, p_small, ps_pt, ps_mm, ps_small),
                  consts, aps)

        # ALL pref loads upfront (before any XPOSE emission)
        _emit_pref_loads(cx, 0, skip=HT)
        for _g in range(1, NGROUPS):
            _emit_pref_loads(cx, _g)

        # c cast (DVE) + group-0/1 prep
        nc.vector.tensor_copy(out=cg16[:], in_=c32a[:])
        _emit_ct(cx, 0)
        _emit_xbar(cx, 0)
        _emit_pe_T(cx, 0)
        _emit_ct(cx, 1)

        # software pipeline, coarse blocks:
        #   iter g: [xbar g+1] [pe-T g+1] [dots+softmax g]
        #           [wsum g-1] [W_MAT g] [cT g+2]
        pend = None
        for g in range(NGROUPS):
            if g + 1 < NGROUPS:
                _emit_xbar(cx, g + 1)
                _emit_pe_T(cx, g + 1)
            st = _phase_dots(cx, g)
            if pend is not None:
                _phase_c(cx, pend)
            _phase_b2(cx, st)
            if g + 2 < NGROUPS:
                _emit_ct(cx, g + 2)
            pend = st

        _phase_c(cx, pend)

    return nc


_NC_CACHE = None
LAST_RESULT = None


def kernel(pref, c, t_pref, t_c):
    global _NC_CACHE, LAST_RESULT
    if _NC_CACHE is None:
        _NC_CACHE = _build_nc()
    nc = _NC_CACHE

    pref = np.ascontiguousarray(pref, dtype=np.float32)
    c = np.ascontiguousarray(c, dtype=np.float32)
    t_pref = np.ascontiguousarray(t_pref, dtype=np.float32)
    t_c = np.ascontiguousarray(t_c, dtype=np.float32)

    in_maps = []
    for i in range(NCORES):
        s = slice(i * BPC, (i + 1) * BPC)
        in_maps.append(
            {"pref": pref[s], "c": c[s], "t_pref": t_pref[s], "t_c": t_c[s]}
        )

    res = run_bass_kernel_spmd(nc, in_maps, list(range(NCORES)))
    LAST_RESULT = res
    return np.ascontiguousarray(
        np.concatenate([r["u"].T for r in res.results], axis=0)
    ).reshape(B, 1, D)


# revision 8
# speedup vs baseline: 1.0246x; 1.0246x over previous
"""Trainium2 Bass kernel for the AggregateLayer pooling problem.

reference semantics (per batch b):
    dot_w[j] = <pref[b,j,:], c[b,0,:]>                      (j = 0..63)
    t_w[j]   = 1 / |t_pref[b,0,j] - t_c[b,0]|
    w        = softmax(dot_w + t_w)                          (over j)
    u[b,0,:] = sum_j w[j] * pref[b,j,:]

Strategy: pure data parallel over 8 NeuronCores (1024 batches each),
batches in groups of GROUP=128 (64 two-batch tiles of 128 flattened
(batch, j) rows x 128 D cols).

Engine budget (measured): pref stream on the gpsimd SWDGE ring runs at
~712 GB/s (47 us); PE tiny matmuls (dot + weighted-sum, N=2) pipeline
their weight loads and cost ~36 ns each (37 us for all 1024); PE
transposes cost ~95 ns each; the XBAR dma-transpose (sync HWDGE) moves
a 128x128 fp16 tile in ~172 ns but is a single shared resource --
concurrent XPOSEs from two HWDGE queues corrupt each other's output
(measured), so ONLY the sync queue ever issues them, and XPOSE
destinations must be contiguous per partition.

The per-group transpose work (needed only to compute the dots) is
split between the XBAR (chunks 0..XCH-1) and PE+PSUM-copy (chunks
XCH..7), balancing PE ~57 us vs XBAR ~55 us. The pipeline runs the
transposes of group g+1 (both paths) during group g's dots/softmax/
weighted-sum, pref loads two groups ahead, and the weighted sum one
group behind, so no engine queue waits mid-stream. u is stored
transposed [D, BPC] straight from the weighted-sum PSUM layout; the
host transposes it back.
"""

import numpy as np
from contextlib import ExitStack

import concourse.bass as bass
import concourse.tile as tile
from concourse import mybir
from concourse.masks import make_identity
from concourse.bass_utils import run_bass_kernel_spmd
import concourse.bass2jax as _b2j


def _split_multiwait(bir: dict) -> int:
    """Walrus in this container rejects >1 sync-wait per instruction.

    Hoist excess waits onto NoOps inserted just before the instruction on
    the same engine (program order within the engine stream preserves the
    wait semantics exactly).
    """
    n = 0
    for fn in bir["functions"]:
        for blk in fn["blocks"]:
            out = []
            for inst in blk["instructions"]:
                si = inst.get("sync_info")
                waits = si.get("on_wait") if si else None
                if waits and len(waits) > 1:
                    for w in waits[:-1]:
                        out.append(
                            {
                                "opcode": "NoOp",
                                "engine": inst["engine"],
                                "name": f"{inst['name']}-xw{n}",
                                "ins": [],
                                "outs": [],
                                "sync_info": {"on_update": [], "on_wait": [w]},
                            }
                        )
                        n += 1
                    si["on_wait"] = [waits[-1]]
                out.append(inst)
            blk["instructions"] = out
    return n


_orig_compile_bir_kernel = _b2j.compile_bir_kernel


def _legalizing_compile_bir_kernel(ant_bir_str, *args, **kwargs):
    import orjson

    bir = orjson.loads(ant_bir_str)
    _split_multiwait(bir)
    return _orig_compile_bir_kernel(orjson.dumps(bir), *args, **kwargs)


_b2j.compile_bir_kernel = _legalizing_compile_bir_kernel

F32 = mybir.dt.float32
F16 = mybir.dt.float16
Alu = mybir.AluOpType
Act = mybir.ActivationFunctionType
Axis = mybir.AxisListType

B, N, D = 8192, 64, 128
NCORES = 8
BPC = B // NCORES          # 1024 batches per core
GROUP = 128                # batches per group
NGROUPS = BPC // GROUP     # 8
NTILES = GROUP // 2        # 64 two-batch tiles per group
NPAIR = GROUP // 2         # 64 batch-pairs per group (softmax partitions)
import os as _os

CH = 8                     # tiles per transpose/copy/dot chunk
NCH = NTILES // CH         # 8 chunks per group
XCH = int(_os.environ.get("K_XCH", "5"))  # chunks via XBAR (rest via PE)
XLAT = float(_os.environ.get("K_XLAT", "0"))  # XPOSE clock stagger (us/group)
HT = 16                    # tiles per pref DMA chunk


class _St:
    """Per-group pipeline state carried between build phases."""

    def __init__(self, g):
        self.g = g
        self.p16 = None
        self.pts = None
        self.tw = None
        self.w = None
        self.nmx = None
        self.wn16 = None
        self.wmat16 = None


class _Ctx:
    def __init__(self, tc, pools, consts, aps):
        self.tc = tc
        self.nc = tc.nc
        (self.p_p16, self.p_pts, self.p_small, self.ps_pt, self.ps_mm,
         self.ps_small) = pools
        self.ident16, self.ident32 = consts
        (self.pref_rows, self.u_all, self.ct16a, self.tpa, self.tca,
         self.cg16, self.wmat_ring, self.p16s, self.ptss) = aps


def _emit_pref_loads(cx, g, skip=0):
    """pref chunk DMAs for group g on the gpsimd SWDGE (fp32->fp16)."""
    nc = cx.nc
    r0 = g * GROUP * N
    for h0 in range(skip, NTILES, HT):
        rh = r0 + h0 * 128
        nc.gpsimd.dma_start(
            out=cx.p16s[g][:, h0 : h0 + HT, :],
            in_=cx.pref_rows[rh : rh + HT * 128, :].rearrange(
                "(t p) d -> p t d", p=128
            ),
        )


def _emit_xbar(cx, g):
    """XBAR transposes for chunks 0..XCH-1 of group g (sync queue ONLY).

    Destination slices are contiguous per partition (full-width innermost
    slices of the pts tile) -- required for correct XPOSE output. One big
    XPOSE per group: fewer HWDGE instructions entangle less with the
    SWDGE vector-clock waits (each extra XPOSE adds spurious cross-ring
    ordering edges on later gpsimd loads).
    """
    nc = cx.nc
    if XCH == 0:
        return
    nt = XCH * CH
    if XLAT > 0:
        with cx.tc.tile_wait_until(XLAT * (g + 1) / 1000.0):
            nc.sync.dma_start_transpose(
                out=cx.ptss[g][:, 0:nt, :],
                in_=cx.p16s[g][:, 0:nt, :].rearrange("p t d -> p (t d)"),
            )
    else:
        nc.sync.dma_start_transpose(
            out=cx.ptss[g][:, 0:nt, :],
            in_=cx.p16s[g][:, 0:nt, :].rearrange("p t d -> p (t d)"),
        )


def _emit_pe_T(cx, g):
    """PE transposes for chunks XCH..NCH-1 of group g + PSUM->SBUF copies."""
    nc = cx.nc
    for idx, k in enumerate(range(XCH, NCH)):
        t0 = k * CH
        pt_ps = cx.ps_pt.tile(
            [128, CH, 128], F16, tag="pt_ps", name=f"ptps{g}_{k}"
        )
        for i in range(CH):
            nc.tensor.transpose(
                out=pt_ps[:, i, :],
                in_=cx.p16s[g][:, t0 + i, :],
                identity=cx.ident16[:],
            )
        if idx % 8 < 3:
            nc.vector.tensor_copy(
                out=cx.ptss[g][:, t0 : t0 + CH, :], in_=pt_ps[:]
            )
        else:
            nc.scalar.copy(out=cx.ptss[g][:, t0 : t0 + CH, :], in_=pt_ps[:])


def _phase_dots(cx, g):
    """tw + dot matmuls from pts + extraction + add/max + softmax tail."""
    nc = cx.nc
    st = _St(g)
    st.p16 = cx.p16s[g]
    st.pts = cx.ptss[g]

    st.tw = cx.p_small.tile([NPAIR, 2, N], F32, tag="tw", name=f"tw{g}")
    for s in range(2):
        nc.vector.tensor_scalar_sub(
            out=st.tw[:, s, :],
            in0=cx.tpa[:, g, s, :],
            scalar1=cx.tca[:, g, s : s + 1],
        )
    nc.scalar.activation(out=st.tw[:], in_=st.tw[:], func=Act.Abs)
    nc.vector.reciprocal(out=st.tw[:], in_=st.tw[:])

    ps_dots = cx.ps_mm.tile(
        [128, NTILES, 2], F32, tag="mm_ps", name=f"dots{g}"
    )
    for k in range(NCH):
        t0 = k * CH
        for i in range(CH):
            t = t0 + i
            nc.tensor.matmul(
                out=ps_dots[:, t, :],
                lhsT=st.pts[:, t, :],
                rhs=cx.ct16a[:, g, 2 * t : 2 * t + 2],
                start=(i == 0),
                stop=(i == CH - 1),
            )

    # valid dots sit at [row, parity=row//64]: extract the two halves
    dotw = cx.p_small.tile([128, NTILES], F32, tag="dotw", name=f"dotw{g}")
    nc.scalar.copy(out=dotw[0:64, :], in_=ps_dots[0:64, :, 0])
    nc.scalar.copy(out=dotw[64:128, :], in_=ps_dots[64:128, :, 1])

    # transpose [128(row), nt] -> [nt, 128(row)] => pair-major dots
    dr_ps = cx.ps_small.tile([NPAIR, 128], F32, tag="sm_ps", name=f"dr{g}")
    nc.tensor.transpose(out=dr_ps[:], in_=dotw[:], identity=cx.ident32[:])

    st.w = cx.p_small.tile([NPAIR, 2, N], F32, tag="w", name=f"w{g}")
    nc.vector.tensor_add(
        out=st.w[:],
        in0=dr_ps[:].rearrange("t (two n) -> t two n", two=2),
        in1=st.tw[:],
    )
    st.nmx = cx.p_small.tile([NPAIR, 2], F32, tag="nmx", name=f"nmx{g}")
    nc.vector.tensor_reduce(
        out=st.nmx[:], in_=st.w[:], axis=Axis.X, op=Alu.max, negate=True
    )
    _phase_b1(cx, st)
    return st


def _phase_b1(cx, st):
    """Softmax tail: exp + sum + reciprocal + normalize (no PE)."""
    nc = cx.nc
    g = st.g
    e = cx.p_small.tile([NPAIR, 2, N], F32, tag="e", name=f"e{g}")
    for s in range(2):
        nc.scalar.activation(
            out=e[:, s, :],
            in_=st.w[:, s, :],
            func=Act.Exp,
            bias=st.nmx[:, s : s + 1],
            scale=1.0,
        )
    z = cx.p_small.tile([NPAIR, 2], F32, tag="z", name=f"z{g}")
    nc.vector.reduce_sum(out=z[:], in_=e[:], axis=Axis.X)
    rz = cx.p_small.tile([NPAIR, 2], F32, tag="rz", name=f"rz{g}")
    nc.vector.reciprocal(out=rz[:], in_=z[:])
    st.wn16 = cx.p_small.tile([NPAIR, 2, N], F16, tag="wn16", name=f"wn{g}")
    for s in range(2):
        nc.vector.tensor_scalar_mul(
            out=st.wn16[:, s, :], in0=e[:, s, :], scalar1=rz[:, s : s + 1]
        )


def _phase_b2(cx, st):
    """W_MAT build: PE transpose of wn16 + block scatter."""
    nc = cx.nc
    g = st.g
    wc_ps = cx.ps_small.tile([128, NTILES], F16, tag="sm_ps", name=f"wc{g}")
    nc.tensor.transpose(
        out=wc_ps[:],
        in_=st.wn16[:].rearrange("t two n -> t (two n)"),
        identity=cx.ident16[0:NPAIR, 0:NPAIR],
    )
    wcol = cx.p_small.tile([128, NTILES], F16, tag="wcol", name=f"wcol{g}")
    nc.vector.tensor_copy(out=wcol[:], in_=wc_ps[:])
    # persistent pre-zeroed ring: only the data halves are ever written,
    # the zero halves survive across generations
    st.wmat16 = cx.wmat_ring[g % len(cx.wmat_ring)]
    nc.vector.tensor_copy(out=st.wmat16[0:64, :, 0], in_=wcol[0:64, :])
    nc.vector.tensor_copy(out=st.wmat16[64:128, :, 1], in_=wcol[64:128, :])


def _phase_c(cx, st):
    """Weighted-sum matmuls + u extraction (DVE) + store (gpsimd)."""
    nc = cx.nc
    g = st.g
    b0 = g * GROUP
    HB = NTILES // 2
    for h in range(2):
        ps_ut = cx.ps_mm.tile(
            [128, HB, 2], F32, tag="mm_ps", name=f"ut{g}_{h}"
        )
        for k in range(HB):
            t = h * HB + k
            nc.tensor.matmul(
                out=ps_ut[:, k, :],
                lhsT=st.p16[:, t, :],
                rhs=st.wmat16[:, t, :],
                start=(k == 0),
                stop=(k == HB - 1),
            )
        uts = cx.p_small.tile(
            [128, GROUP // 2], F32, tag="uts", name=f"uts{g}_{h}"
        )
        nc.vector.tensor_copy(
            out=uts[:], in_=ps_ut[:].rearrange("d t two -> d (t two)")
        )
        bh = b0 + h * (GROUP // 2)
        nc.gpsimd.dma_start(
            out=cx.u_all[:, bh : bh + GROUP // 2], in_=uts[:]
        )


def _emit_ct(cx, g):
    """PE transpose of group g's c half into ct16a (+ DVE copy)."""
    nc = cx.nc
    ct_ps = cx.ps_small.tile([128, 128], F16, tag="sm_ps", name=f"ct{g}")
    nc.tensor.transpose(
        out=ct_ps[:],
        in_=cx.cg16[:, g, :],
        identity=cx.ident16[:],
    )
    nc.vector.tensor_copy(out=cx.ct16a[:, g, :], in_=ct_ps[:])


def _build_nc():
    nc = bass.Bass()
    pref = nc.declare_dram_parameter("pref", [BPC, N, D], F32, isOutput=False)
    c = nc.declare_dram_parameter("c", [BPC, 1, D], F32, isOutput=False)
    t_pref = nc.declare_dram_parameter("t_pref", [BPC, 1, N], F32, isOutput=False)
    t_c = nc.declare_dram_parameter("t_c", [BPC, 1], F32, isOutput=False)
    # u stored transposed [D, BPC] (direct from the weighted-sum PSUM
    # layout); the host transposes it back.
    u = nc.declare_dram_parameter("u", [D, BPC], F32, isOutput=True)

    pref_rows = pref[:].rearrange("b n d -> (b n) d")
    c_all = c[:].rearrange("b one d -> (b one) d")
    tp_all = t_pref[:].rearrange("b one n -> (b one) n")
    tc_all = t_c[:]
    u_all = u[:]

    with ExitStack() as ctx:
        tc = ctx.enter_context(tile.TileContext(nc))
        p_const = ctx.enter_context(tc.tile_pool(name="const", bufs=1))
        p_pre = ctx.enter_context(tc.tile_pool(name="pre", bufs=1))
        # bufs=8: every group keeps its own p16 slot for the whole run.
        # No ring reuse -> the pref loads carry ZERO WAR edges, and since
        # they are all emitted before any XPOSE they cannot inherit the
        # scheduler's conservative cross-ring (SWDGE<->HWDGE) waits that
        # serialized the pipeline when loads trailed XPOSEs.
        p_p16 = ctx.enter_context(tc.tile_pool(name="p16", bufs=8))
        p_pts = ctx.enter_context(tc.tile_pool(name="pts", bufs=3))
        p_small = ctx.enter_context(tc.tile_pool(name="small", bufs=3))
        ps_pt = ctx.enter_context(tc.tile_pool(name="ps_pt", bufs=3, space="PSUM"))
        ps_mm = ctx.enter_context(tc.tile_pool(name="ps_mm", bufs=3, space="PSUM"))
        ps_small = ctx.enter_context(
            tc.tile_pool(name="ps_small", bufs=2, space="PSUM")
        )

        nb = NGROUPS * GROUP

        # persistent rings
        p16s = []
        ptss = []
        for _gi in range(NGROUPS):
            p16s.append(
                p_p16.tile([128, NTILES, D], F16, tag="p16", name=f"p16_{_gi}")
            )
            ptss.append(
                p_pts.tile([128, NTILES, D], F16, tag="pts", name=f"pts_{_gi}")
            )

        # gpsimd stream head: group 0's first pref chunk, then identity
        # builds (gpsimd-only affine_select), then c/t loads, then the
        # rest of pref.
        nc.gpsimd.dma_start(
            out=p16s[0][:, 0:HT, :],
            in_=pref_rows[0 : HT * 128, :].rearrange("(t p) d -> p t d", p=128),
        )
        ident16 = p_const.tile([128, 128], F16)
        make_identity(nc, ident16[:])

        c32a = p_pre.tile([128, NGROUPS, D], F32)
        nc.gpsimd.dma_start(
            out=c32a[:],
            in_=c_all[0:nb, :].rearrange("(g b) d -> b g d", b=128),
        )
        tpa = p_pre.tile([NPAIR, NGROUPS, 2, N], F32)
        nc.gpsimd.dma_start(
            out=tpa[:],
            in_=tp_all[0:nb, :].rearrange(
                "(g t two) n -> t g two n", t=NPAIR, two=2
            ),
        )
        tca = p_pre.tile([NPAIR, NGROUPS, 2], F32)
        nc.gpsimd.dma_start(
            out=tca[:],
            in_=tc_all[0:nb, :].rearrange(
                "(g t two) one -> t g (two one)", t=NPAIR, two=2
            ),
        )
        ident32 = p_const.tile([128, 128], F32)
        make_identity(nc, ident32[:])
        consts = (ident16, ident32)

        _emit_pref_loads_head = True
        # rest of group 0 + group 1 pref
        # (emitted below via cx; build cx first)
        cg16 = p_pre.tile([128, NGROUPS, D], F16)
        ct16a = p_pre.tile([128, NGROUPS, 128], F16)  # [D, group, batch]

        wmat_ring = []
        for _wi in range(3):
            wm = p_pre.tile([128, NTILES, 2], F16, name=f"wmatr{_wi}")
            nc.vector.memset(wm[:], 0.0)
            wmat_ring.append(wm)

        aps = (pref_rows, u_all, ct16a, tpa, tca, cg16, wmat_ring, p16s, ptss)
        cx = _Ctx(tc, (p_p16, p_pts, p_small, ps_pt, ps_mm, ps_small),
                  consts, aps)

        # ALL pref loads upfront (before any XPOSE emission)
        _emit_pref_loads(cx, 0, skip=HT)
        for _g in range(1, NGROUPS):
            _emit_pref_loads(cx, _g)

        # c cast (DVE) + group-0/1 prep
        nc.vector.tensor_copy(out=cg16[:], in_=c32a[:])
        _emit_ct(cx, 0)
        _emit_xbar(cx, 0)
        _emit_pe_T(cx, 0)
        _emit_ct(cx, 1)

        # software pipeline, coarse blocks:
        #   iter g: [xbar g+1] [pe-T g+1] [dots+softmax g]
        #           [wsum g-1] [W_MAT g] [cT g+2]
        pend = None
        for g in range(NGROUPS):
            if g + 1 < NGROUPS:
                _emit_xbar(cx, g + 1)
                _emit_pe_T(cx, g + 1)
            st = _phase_dots(cx, g)
            if pend is not None:
                _phase_c(cx, pend)
            _phase_b2(cx, st)
            if g + 2 < NGROUPS:
                _emit_ct(cx, g + 2)
            pend = st

        _phase_c(cx, pend)

    return nc


_NC_CACHE = None
LAST_RESULT = None


def kernel(pref, c, t_pref, t_c):
    global _NC_CACHE, LAST_RESULT
    if _NC_CACHE is None:
        _NC_CACHE = _build_nc()
    nc = _NC_CACHE

    pref = np.ascontiguousarray(pref, dtype=np.float32)
    c = np.ascontiguousarray(c, dtype=np.float32)
    t_pref = np.ascontiguousarray(t_pref, dtype=np.float32)
    t_c = np.ascontiguousarray(t_c, dtype=np.float32)

    in_maps = []
    for i in range(NCORES):
        s = slice(i * BPC, (i + 1) * BPC)
        in_maps.append(
            {"pref": pref[s], "c": c[s], "t_pref": t_pref[s], "t_c": t_c[s]}
        )

    res = run_bass_kernel_spmd(nc, in_maps, list(range(NCORES)))
    LAST_RESULT = res
    return np.ascontiguousarray(
        np.concatenate([r["u"].T for r in res.results], axis=0)
    ).reshape(B, 1, D)


# revision 13
# speedup vs baseline: 1.5976x; 1.5593x over previous
"""Trainium2 Bass kernel for the AggregateLayer pooling problem.

reference semantics (per batch b):
    dot_w[j] = <pref[b,j,:], c[b,0,:]>                      (j = 0..63)
    t_w[j]   = 1 / |t_pref[b,0,j] - t_c[b,0]|
    w        = softmax(dot_w + t_w)                          (over j)
    u[b,0,:] = sum_j w[j] * pref[b,j,:]

Strategy: pure data parallel over 8 NeuronCores (1024 batches each).

SPLIT ARCHITECTURE (v2): the 1024 batches per core are split 768/256
between two dataflows so that no single engine carries the whole
problem:

 * ROW PATH (groups 0..5, batches 0..767): the measured-tuned pipeline
   from the 136us baseline -- tiles of 128 flattened (batch, j) rows x
   128 D cols; PE transposes each tile (needed only for the dot
   products), tiny N=2 dot and weighted-sum matmuls (36 ns each,
   weight loads pipeline), PSUM->SBUF copies on ACT, softmax on
   ACT/DVE. PE busy ~11 us/group.
 * BATCH PATH (batches 768..1023, two 128-batch blocks): batch index
   on partitions, (j, d) on the free dim; dots, softmax and the
   weighted sum are all WITHIN-partition ops on DVE/ACT with stride-0
   broadcast APs and fp16 tree reductions -- zero PE, zero PSUM.
   ~22 us DVE/block, interleaved into the row iterations' DVE idle.

Engine budget: PE ~67us, DVE ~59us, ACT ~64us, gpsimd ~49us.

Measured facts this build relies on (do not "fix" without re-measuring):
 - tiny N=2 matmuls back-to-back cost ~36 ns (LDWEIGHTS hides); PE
   transposes ~95 ns; the pref SWDGE stream runs at ~712 GB/s.
 - the XBAR dma-transpose CANNOT be used concurrently with SWDGE bulk
   loads: the runtime serializes HWDGE-XPOSE against SWDGE with
   ~2-12us ring-handoff penalties (and concurrent XPOSEs on the two
   HWDGE queues corrupt output), so it loses to PE transposes.
 - DVE tensor_tensor with an outer-broadcast (stride-0 over the middle
   dim) fp16 operand runs at 2 elem/lane/cycle; with an inner
   (stride-0 innermost) broadcast it drops to 1x; tensor_reduce also
   runs at 1x, so fp16 tree-adds (2x) replace it where possible.
 - regular sync-HWDGE DMAs (c/t loads, u stores) coexist fine with the
   gpsimd SWDGE pref stream (baseline-proven).
 - the last row group's HT=8 DMA chunking is load-bearing (HT=16 there
   reproducibly ~2x-es runtime via p16 buffer-ring interaction).
Run-to-run noise is +-3 us with occasional +15-40 us device-state
excursions -- attribute changes only across repeated runs.
"""

import numpy as np
from contextlib import ExitStack

import concourse.bass as bass
import concourse.tile as tile
from concourse import mybir
from concourse.masks import make_identity
from concourse.bass_utils import run_bass_kernel_spmd
import concourse.bass2jax as _b2j


def _split_multiwait(bir: dict) -> int:
    """Walrus in this container rejects >1 sync-wait per instruction.

    Hoist excess waits onto NoOps inserted just before the instruction on
    the same engine (program order within the engine stream preserves the
    wait semantics exactly).
    """
    n = 0
    for fn in bir["functions"]:
        for blk in fn["blocks"]:
            out = []
            for inst in blk["instructions"]:
                si = inst.get("sync_info")
                waits = si.get("on_wait") if si else None
                if waits and len(waits) > 1:
                    for w in waits[:-1]:
                        out.append(
                            {
                                "opcode": "NoOp",
                                "engine": inst["engine"],
                                "name": f"{inst['name']}-xw{n}",
                                "ins": [],
                                "outs": [],
                                "sync_info": {"on_update": [], "on_wait": [w]},
                            }
                        )
                        n += 1
                    si["on_wait"] = [waits[-1]]
                out.append(inst)
            blk["instructions"] = out
    return n


_orig_compile_bir_kernel = _b2j.compile_bir_kernel


def _legalizing_compile_bir_kernel(ant_bir_str, *args, **kwargs):
    import orjson

    bir = orjson.loads(ant_bir_str)
    _split_multiwait(bir)
    return _orig_compile_bir_kernel(orjson.dumps(bir), *args, **kwargs)


_b2j.compile_bir_kernel = _legalizing_compile_bir_kernel

F32 = mybir.dt.float32
F16 = mybir.dt.float16
Alu = mybir.AluOpType
Act = mybir.ActivationFunctionType
Axis = mybir.AxisListType

B, N, D = 8192, 64, 128
NCORES = 8
BPC = B // NCORES          # 1024 batches per core
GROUP = 128                # batches per group
NGR = 6                    # row-path groups (batches 0..767)
NBG = 2                    # batch-path blocks (batches 768..1023)
NB_ROW = NGR * GROUP       # 768
NTILES = GROUP // 2        # 64 two-batch tiles per group
NPAIR = GROUP // 2         # 64 batch-pairs per group (softmax partitions)
CH = 8                     # tiles per transpose/copy chunk
NCH = NTILES // CH         # 8 chunks per group
LAG = 3                    # dot-chunk k trails transpose-chunk k+LAG


class _St:
    """Per-group pipeline state carried between build phases."""

    def __init__(self, g):
        self.g = g
        self.p16 = None
        self.tw = None
        self.w = None
        self.nmx = None
        self.wn16 = None
        self.wmat16 = None


class _Ctx:
    def __init__(self, tc, pools, consts, aps):
        self.tc = tc
        self.nc = tc.nc
        (self.p_p16, self.p_pt, self.p_small, self.ps_pt, self.ps_mm,
         self.ps_small) = pools
        self.ident16, self.ident32 = consts
        (self.pref_rows, self.u_all, self.ct16a, self.tpa, self.ntca,
         self.cg16, self.wmat_ring) = aps
        self.bg_steps = []


def _phase_a(cx, g, p16):
    """tw + interleaved transpose/dot passes + dots extraction + add/max."""
    nc = cx.nc
    st = _St(g)
    st.p16 = p16

    # pref chunk DMAs for this group (group 0's first chunk was emitted
    # ahead of the identity build in _build_nc); last group uses smaller
    # chunks so the drain after the final chunk is short
    r0 = g * GROUP * N
    HT = 16 if g < NGR - 1 else 8
    for h0 in range(16 if g == 0 else 0, NTILES, HT):
        rh = r0 + h0 * 128
        nc.gpsimd.dma_start(
            out=p16[:, h0 : h0 + HT, :],
            in_=cx.pref_rows[rh : rh + HT * 128, :].rearrange(
                "(t p) d -> p t d", p=128
            ),
        )

    # t_w = 1/|t_pref - t_c| entirely on ACT (bias = -t_c per partition)
    st.tw = cx.p_small.tile([NPAIR, 2, N], F32, tag="tw", name=f"tw{g}")
    for s in range(2):
        nc.scalar.activation(
            out=st.tw[:, s, :],
            in_=cx.tpa[:, g, s, :],
            func=Act.Abs,
            bias=cx.ntca[:, g, s : s + 1],
            scale=1.0,
        )
    nc.vector.reciprocal(out=st.tw[:], in_=st.tw[:])

    pts = cx.p_pt.tile([128, NTILES, 128], F16, tag="pts", name=f"pts{g}")
    ps_dots = cx.ps_mm.tile(
        [128, NTILES, 2], F32, tag="mm_ps", name=f"dots{g}"
    )

    def t_chunk(k):
        t0 = k * CH
        pt_ps = cx.ps_pt.tile(
            [128, CH, 128], F16, tag="pt_ps", name=f"ptps{g}_{k}"
        )
        for i in range(CH):
            nc.tensor.transpose(
                out=pt_ps[:, i, :],
                in_=p16[:, t0 + i, :],
                identity=cx.ident16[:],
            )
        nc.scalar.copy(out=pts[:, t0 : t0 + CH, :], in_=pt_ps[:])

    def d_chunk(k):
        t0 = k * CH
        for i in range(CH):
            t = t0 + i
            th, tr = divmod(t, 128 // 2)
            nc.tensor.matmul(
                out=ps_dots[:, t, :],
                lhsT=pts[:, t, :],
                rhs=cx.ct16a[:, g + th, 2 * tr : 2 * tr + 2],
                start=(i == 0),
                stop=(i == CH - 1),
            )

    for k in range(NCH):
        t_chunk(k)
        if k >= LAG:
            d_chunk(k - LAG)
    for k in range(NCH - LAG, NCH):
        d_chunk(k)

    # valid dots sit at [row, parity=row//64]: extract the two halves
    dotw = cx.p_small.tile([128, NTILES], F32, tag="dotw", name=f"dotw{g}")
    nc.scalar.copy(out=dotw[0:64, :], in_=ps_dots[0:64, :, 0])
    nc.scalar.copy(out=dotw[64:128, :], in_=ps_dots[64:128, :, 1])

    # transpose [128(row), nt] -> [nt, 128(row)] => pair-major dots
    dr_ps = cx.ps_small.tile([NPAIR, 128], F32, tag="sm_ps", name=f"dr{g}")
    nc.tensor.transpose(out=dr_ps[:], in_=dotw[:], identity=cx.ident32[:])

    st.w = cx.p_small.tile([NPAIR, 2, N], F32, tag="w", name=f"w{g}")
    nc.vector.tensor_add(
        out=st.w[:],
        in0=dr_ps[:].rearrange("t (two n) -> t two n", two=2),
        in1=st.tw[:],
    )
    st.nmx = cx.p_small.tile([NPAIR, 2], F32, tag="nmx", name=f"nmx{g}")
    nc.vector.tensor_reduce(
        out=st.nmx[:], in_=st.w[:], axis=Axis.X, op=Alu.max, negate=True
    )
    _phase_b1(cx, st)
    return st


def _phase_b1(cx, st):
    """Softmax tail: exp (ACT) + sum (DVE) + recip+normalize (ACT)."""
    nc = cx.nc
    g = st.g
    e = cx.p_small.tile([NPAIR, 2, N], F32, tag="e", name=f"e{g}")
    for s in range(2):
        nc.scalar.activation(
            out=e[:, s, :],
            in_=st.w[:, s, :],
            func=Act.Exp,
            bias=st.nmx[:, s : s + 1],
            scale=1.0,
        )
    z = cx.p_small.tile([NPAIR, 2], F32, tag="z", name=f"z{g}")
    nc.vector.reduce_sum(out=z[:], in_=e[:], axis=Axis.X)
    rz = cx.p_small.tile([NPAIR, 2], F32, tag="rz", name=f"rz{g}")
    nc.vector.reciprocal(out=rz[:], in_=z[:])
    st.wn16 = cx.p_small.tile([NPAIR, 2, N], F16, tag="wn16", name=f"wn{g}")
    for s in range(2):
        nc.scalar.activation(
            out=st.wn16[:, s, :],
            in_=e[:, s, :],
            func=Act.Copy,
            scale=rz[:, s : s + 1],
        )


def _phase_b2(cx, st):
    """W_MAT build: PE transpose of wn16 + block scatter."""
    nc = cx.nc
    g = st.g
    wc_ps = cx.ps_small.tile([128, NTILES], F16, tag="sm_ps", name=f"wc{g}")
    nc.tensor.transpose(
        out=wc_ps[:],
        in_=st.wn16[:].rearrange("t two n -> t (two n)"),
        identity=cx.ident16[0:NPAIR, 0:NPAIR],
    )
    wcol = cx.p_small.tile([128, NTILES], F16, tag="wcol", name=f"wcol{g}")
    nc.vector.tensor_copy(out=wcol[:], in_=wc_ps[:])
    # persistent pre-zeroed ring: only the data halves are ever written,
    # the zero halves survive across generations
    st.wmat16 = cx.wmat_ring[g % len(cx.wmat_ring)]
    nc.vector.tensor_copy(out=st.wmat16[0:64, :, 0], in_=wcol[0:64, :])
    nc.vector.tensor_copy(out=st.wmat16[64:128, :, 1], in_=wcol[64:128, :])


def _phase_c(cx, st):
    """Weighted-sum matmuls + u extraction (DVE) + store (sync)."""
    nc = cx.nc
    g = st.g
    b0 = g * GROUP
    HB = NTILES // 2
    for h in range(2):
        ps_ut = cx.ps_mm.tile(
            [128, HB, 2], F32, tag="mm_ps", name=f"ut{g}_{h}"
        )
        for k in range(HB):
            t = h * HB + k
            nc.tensor.matmul(
                out=ps_ut[:, k, :],
                lhsT=st.p16[:, t, :],
                rhs=st.wmat16[:, t, :],
                start=(k == 0),
                stop=(k == HB - 1),
            )
        uts = cx.p_small.tile(
            [128, GROUP // 2], F32, tag="uts", name=f"uts{g}_{h}"
        )
        nc.vector.tensor_copy(
            out=uts[:], in_=ps_ut[:].rearrange("d t two -> d (t two)")
        )
        bh = b0 + h * (GROUP // 2)
        nc.sync.dma_start(
            out=cx.u_all[:, bh : bh + GROUP // 2], in_=uts[:]
        )


def _emit_ct(cx, g):
    """PE transposes of group g's c half into ct16a (+ copies)."""
    nc = cx.nc
    ct_ps = cx.ps_small.tile([128, 128], F16, tag="sm_ps", name=f"ct{g}")
    nc.tensor.transpose(
        out=ct_ps[:],
        in_=cx.cg16[:, g, :],
        identity=cx.ident16[:],
    )
    nc.vector.tensor_copy(out=cx.ct16a[:, g, :], in_=ct_ps[:])


def _bg_build_steps(cx, p, bi, p16b, c16b, tpb, ntcb, ub_all):
    """Batch-path (batches on partitions) step closures for block bi.

    All within-partition DVE/ACT work; no PE, no PSUM. Steps are emitted
    interleaved into the row iterations so the long DVE ops fill idle
    time without blocking the row softmax chain.
    """
    nc = cx.nc
    buf = p.tile([128, N, D], F16, tag="bgbuf", name=f"bgbuf{bi}")
    dots = p.tile([128, N], F32, tag="bgd", name=f"bgd{bi}")
    tw = p.tile([128, N], F32, tag="bgtw", name=f"bgtw{bi}")
    w = p.tile([128, N], F32, tag="bgw", name=f"bgw{bi}")
    nmx = p.tile([128, 1], F32, tag="bgn", name=f"bgn{bi}")
    e = p.tile([128, N], F32, tag="bge", name=f"bge{bi}")
    z = p.tile([128, 1], F32, tag="bgz", name=f"bgz{bi}")
    rz = p.tile([128, 1], F32, tag="bgrz", name=f"bgrz{bi}")
    wn16 = p.tile([128, N], F16, tag="bgwn", name=f"bgwn{bi}")
    u32 = p.tile([128, D], F32, tag="bgu", name=f"bgu{bi}")

    def s_mult(h):
        def f():
            j0, j1 = (0, N // 2) if h == 0 else (N // 2, N)
            nc.vector.tensor_tensor(
                out=buf[:, j0:j1, :],
                in0=p16b[:, j0:j1, :],
                in1=c16b[:, None, :].broadcast_to([128, j1 - j0, D]),
                op=Alu.mult,
            )
        return f

    def s_tree_d():
        # 3 fp16 tree levels over d: [128, 64, 128] -> [128, 64, 16]
        lvl = D
        while lvl > 16:
            h = lvl // 2
            nc.vector.tensor_tensor(
                out=buf[:, :, 0:h],
                in0=buf[:, :, 0:h],
                in1=buf[:, :, h:lvl],
                op=Alu.add,
            )
            lvl = h

    def s_dots():
        nc.vector.tensor_reduce(
            out=dots[:], in_=buf[:, :, 0:16], axis=Axis.X, op=Alu.add
        )
        nc.scalar.activation(
            out=tw[:], in_=tpb[:], func=Act.Abs, bias=ntcb[:], scale=1.0
        )
        nc.vector.reciprocal(out=tw[:], in_=tw[:])

    def s_softmax():
        nc.vector.tensor_tensor(out=w[:], in0=dots[:], in1=tw[:], op=Alu.add)
        nc.vector.tensor_reduce(
            out=nmx[:], in_=w[:], axis=Axis.X, op=Alu.max, negate=True
        )
        nc.scalar.activation(
            out=e[:], in_=w[:], func=Act.Exp, bias=nmx[:], scale=1.0
        )
        nc.vector.reduce_sum(out=z[:], in_=e[:], axis=Axis.X)
        nc.vector.reciprocal(out=rz[:], in_=z[:])
        nc.scalar.activation(
            out=wn16[:], in_=e[:], func=Act.Copy, scale=rz[:]
        )

    def s_mult2(h):
        def f():
            j0, j1 = (0, N // 2) if h == 0 else (N // 2, N)
            nc.vector.tensor_tensor(
                out=buf[:, j0:j1, :],
                in0=p16b[:, j0:j1, :],
                in1=wn16[:, j0:j1, None].broadcast_to([128, j1 - j0, D]),
                op=Alu.mult,
            )
        return f

    def s_tree_j():
        lvl = N
        while lvl > 2:
            h = lvl // 2
            nc.vector.tensor_tensor(
                out=buf[:, 0:h, :],
                in0=buf[:, 0:h, :],
                in1=buf[:, h:lvl, :],
                op=Alu.add,
            )
            lvl = h

    def s_store():
        nc.vector.tensor_tensor(
            out=u32[:], in0=buf[:, 0, :], in1=buf[:, 1, :], op=Alu.add
        )
        nc.sync.dma_start(
            out=ub_all[bi * 128 : (bi + 1) * 128, :], in_=u32[:]
        )

    return [
        s_mult(0), s_mult(1), s_tree_d, s_dots, s_softmax,
        s_mult2(0), s_mult2(1), s_tree_j, s_store,
    ]


def _build_nc():
    nc = bass.Bass()
    pref = nc.declare_dram_parameter("pref", [BPC, N, D], F32, isOutput=False)
    c = nc.declare_dram_parameter("c", [BPC, 1, D], F32, isOutput=False)
    t_pref = nc.declare_dram_parameter("t_pref", [BPC, 1, N], F32, isOutput=False)
    t_c = nc.declare_dram_parameter("t_c", [BPC, 1], F32, isOutput=False)
    # row-path u stored transposed [D, 768] (direct from the weighted-sum
    # PSUM layout); batch-path ub stored natural [256, D]. The host
    # combines them.
    u = nc.declare_dram_parameter("u", [D, NB_ROW], F32, isOutput=True)
    ub = nc.declare_dram_parameter("ub", [NBG * 128, D], F32, isOutput=True)

    pref_rows = pref[:].rearrange("b n d -> (b n) d")
    c_all = c[:].rearrange("b one d -> (b one) d")
    tp_all = t_pref[:].rearrange("b one n -> (b one) n")
    tc_all = t_c[:]
    u_all = u[:]
    ub_all = ub[:]

    with ExitStack() as ctx:
        tc = ctx.enter_context(tile.TileContext(nc))
        p_const = ctx.enter_context(tc.tile_pool(name="const", bufs=1))
        p_pre = ctx.enter_context(tc.tile_pool(name="pre", bufs=1))
        p_p16 = ctx.enter_context(tc.tile_pool(name="p16", bufs=4))
        p_pt = ctx.enter_context(tc.tile_pool(name="pt", bufs=3))
        p_small = ctx.enter_context(tc.tile_pool(name="small", bufs=3))
        p_bg = ctx.enter_context(tc.tile_pool(name="bg", bufs=1))
        p_bg2 = ctx.enter_context(tc.tile_pool(name="bg2", bufs=2))
        ps_pt = ctx.enter_context(tc.tile_pool(name="ps_pt", bufs=3, space="PSUM"))
        ps_mm = ctx.enter_context(tc.tile_pool(name="ps_mm", bufs=3, space="PSUM"))
        ps_small = ctx.enter_context(
            tc.tile_pool(name="ps_small", bufs=2, space="PSUM")
        )

        # sync stream head: group 0's c first (contiguous -- it gates
        # group 0's dot matmuls), then t tensors, then the rest of c,
        # then the batch-path loads.
        c32a = p_pre.tile([128, NGR, D], F32)
        nc.sync.dma_start(
            out=c32a[:, 0:1, :],
            in_=c_all[0:GROUP, :].rearrange("(h b) d -> b h d", b=128),
        )
        tpa = p_pre.tile([NPAIR, NGR, 2, N], F32)
        nc.sync.dma_start(
            out=tpa[:],
            in_=tp_all[0:NB_ROW, :].rearrange(
                "(g t two) n -> t g two n", t=NPAIR, two=2
            ),
        )
        tca = p_pre.tile([NPAIR, NGR, 2], F32)
        nc.sync.dma_start(
            out=tca[:],
            in_=tc_all[0:NB_ROW, :].rearrange(
                "(g t two) one -> t g (two one)", t=NPAIR, two=2
            ),
        )
        nc.sync.dma_start(
            out=c32a[:, 1:, :],
            in_=c_all[GROUP:NB_ROW, :].rearrange("(g b) d -> b g d", b=128),
        )
        # batch-path loads: the fp32->fp16 casting loads (p16b, c16b)
        # must ride the gpsimd SWDGE (HWDGE cannot cast); they are
        # emitted lazily after row-group bi's chunks so the head is not
        # delayed. tpb/tcb are fp32 passthrough -> sync HWDGE now.
        p16bs = []
        c16bs = []
        tpbs = []
        tcbs = []
        bg_loads = []
        for bi in range(NBG):
            bs = NB_ROW + bi * 128
            p16b = p_bg2.tile([128, N, D], F16, tag="p16b", name=f"p16b{bi}")
            c16b = p_bg2.tile([128, D], F16, tag="c16b", name=f"c16b{bi}")
            tpb = p_bg2.tile([128, N], F32, tag="tpb", name=f"tpb{bi}")
            nc.sync.dma_start(out=tpb[:], in_=tp_all[bs : bs + 128, :])
            tcb = p_bg2.tile([128, 1], F32, tag="tcb", name=f"tcb{bi}")
            nc.sync.dma_start(out=tcb[:], in_=tc_all[bs : bs + 128, :])

            def _ld(bi=bi, bs=bs, p16b=p16b, c16b=c16b):
                nc.gpsimd.dma_start(out=c16b[:], in_=c_all[bs : bs + 128, :])
                nc.gpsimd.dma_start(out=p16b[:], in_=pref[bs : bs + 128, :, :])

            bg_loads.append(_ld)
            p16bs.append(p16b)
            c16bs.append(c16b)
            tpbs.append(tpb)
            tcbs.append(tcb)

        # gpsimd stream: group 0's first pref chunk, then the identity
        # build (gpsimd-only affine_select) -- identities are ready right
        # when chunk 0's data lands.
        p16s = []
        for _gi in range(NGR):
            p16_t = p_p16.tile(
                [128, NTILES, D], F16, tag="p16", name=f"p16_{_gi}"
            )
            p16s.append(p16_t)
        nc.gpsimd.dma_start(
            out=p16s[0][:, 0:16, :],
            in_=pref_rows[0 : 16 * 128, :].rearrange("(t p) d -> p t d", p=128),
        )
        ident16 = p_const.tile([128, 128], F16)
        ident32 = p_const.tile([128, 128], F32)
        make_identity(nc, ident16[:])
        make_identity(nc, ident32[:])
        consts = (ident16, ident32)

        # negated t_c (bias operands for the ACT abs) + c cast
        ntca = p_pre.tile([NPAIR, NGR, 2], F32)
        nc.vector.tensor_scalar_mul(out=ntca[:], in0=tca[:], scalar1=-1.0)
        ntcbs = []
        for bi in range(NBG):
            ntcb = p_bg2.tile([128, 1], F32, tag="ntcb", name=f"ntcb{bi}")
            nc.vector.tensor_scalar_mul(
                out=ntcb[:], in0=tcbs[bi][:], scalar1=-1.0
            )
            ntcbs.append(ntcb)

        cg16 = p_pre.tile([128, NGR, D], F16)
        nc.vector.tensor_copy(out=cg16[:, 0:1, :], in_=c32a[:, 0:1, :])
        nc.vector.tensor_copy(out=cg16[:, 1:, :], in_=c32a[:, 1:, :])
        ct16a = p_pre.tile([128, NGR, 128], F16)  # [D, group, batch]

        wmat_ring = []
        for _wi in range(3):
            wm = p_pre.tile([128, NTILES, 2], F16, name=f"wmatr{_wi}")
            nc.vector.memset(wm[:], 0.0)
            wmat_ring.append(wm)

        aps = (pref_rows, u_all, ct16a, tpa, ntca, cg16, wmat_ring)
        cx = _Ctx(tc, (p_p16, p_pt, p_small, ps_pt, ps_mm, ps_small),
                  consts, aps)

        # batch-path step queue (interleaved into the row iterations)
        for bi in range(NBG):
            cx.bg_steps.extend(
                _bg_build_steps(cx, p_bg, bi, p16bs[bi], c16bs[bi],
                                tpbs[bi], ntcbs[bi], ub_all)
            )

        _emit_ct(cx, 0)

        # software pipeline, coarse blocks (fine interleaving costs ~10ns
        # of semaphore latency per matmul -- measured):
        #   iter g: [softmax-tail g-1] [A: loads+transposes+dots g]
        #           [W_MAT g-1] [weighted-sum + store g-1] [cT g+1]
        #           [~3 batch-path steps]
        nsteps = len(cx.bg_steps)
        pend = None
        for g in range(NGR):
            st = _phase_a(cx, g, p16s[g])
            if g < NBG:
                bg_loads[g]()
            if pend is not None:
                _phase_c(cx, pend)
            _phase_b2(cx, st)
            if g + 1 < NGR:
                _emit_ct(cx, g + 1)
            # drain bg steps evenly: by end of iter g, (g+1)/NGR of them
            want = (nsteps * (g + 1)) // NGR
            while len(cx.bg_steps) > nsteps - want:
                cx.bg_steps.pop(0)()
            pend = st

        _phase_c(cx, pend)

    return nc


_NC_CACHE = None
LAST_RESULT = None


def kernel(pref, c, t_pref, t_c):
    global _NC_CACHE, LAST_RESULT
    if _NC_CACHE is None:
        _NC_CACHE = _build_nc()
    nc = _NC_CACHE

    pref = np.ascontiguousarray(pref, dtype=np.float32)
    c = np.ascontiguousarray(c, dtype=np.float32)
    t_pref = np.ascontiguousarray(t_pref, dtype=np.float32)
    t_c = np.ascontiguousarray(t_c, dtype=np.float32)

    in_maps = []
    for i in range(NCORES):
        s = slice(i * BPC, (i + 1) * BPC)
        in_maps.append(
            {"pref": pref[s], "c": c[s], "t_pref": t_pref[s], "t_c": t_c[s]}
        )

    res = run_bass_kernel_spmd(nc, in_maps, list(range(NCORES)))
    LAST_RESULT = res
    out = np.empty((B, D), dtype=np.float32)
    for i, r in enumerate(res.results):
        b0 = i * BPC
        out[b0 : b0 + NB_ROW] = r["u"].T
        out[b0 + NB_ROW : b0 + BPC] = r["ub"]
    return np.ascontiguousarray(out).reshape(B, 1, D)


# revision 15
# speedup vs baseline: 1.8080x; 1.1317x over previous
"""Trainium2 Bass kernel for the AggregateLayer pooling problem.

reference semantics (per batch b):
    dot_w[j] = <pref[b,j,:], c[b,0,:]>                      (j = 0..63)
    t_w[j]   = 1 / |t_pref[b,0,j] - t_c[b,0]|
    w        = softmax(dot_w + t_w)                          (over j)
    u[b,0,:] = sum_j w[j] * pref[b,j,:]

Strategy: pure data parallel over 8 NeuronCores (1024 batches each).
Per core, batches are processed in groups of GROUP=128 (NTILES=64 tiles
of 2 batches; a tile is the 128 flattened (batch, j) rows x 128 D cols).

The kernel is HBM-bandwidth-bound: the pref stream (33.5 MB fp32/core)
runs at the ~360 GB/s per-core HBM cap (~95 us of pure streaming).
Everything else is engineered to hide under that stream:
  - pref chunk loads (cast fp32->fp16 in SWDGE) are the only traffic on
    the gpsimd ring; c/t_pref/t_c loads and u stores ride sync HWDGE.
  - group 0's c slice loads first (contiguous 64 KB) so its dot matmuls
    aren't gated; later groups' c transposes are emitted lazily; the
    identity build (gpsimd-only) slots behind group 0's first chunk.
  - engine streams are in-order, so emission is software-pipelined at
    COARSE block granularity (fine interleaving measurably costs ~10ns
    of semaphore latency per matmul): dot-matmul chunks trail the
    transpose chunks by LAG=3 (both paced by the PSUM->SBUF pts copies,
    split 5:3 ACT/DVE), and each group's weighted sum is deferred one
    full group so PE chews on group g+1's transposes while group g's
    softmax runs on DVE/ACT.
  - u is stored transposed [D, BPC] straight from the weighted-sum PSUM
    layout (saves two PE transposes + copies per group); the host
    transposes it back. W_MAT lives in a persistent pre-zeroed 3-ring.

Tuning cliffs measured on hardware (do not "fix" these without
re-measuring): the last group's HT=8 DMA chunking is load-bearing
(HT=16 there reproducibly ~2x-es runtime via p16 buffer-ring
interaction); per-half softmax chains, fully-upfront DMA emission, and
fp32-bitcast PSUM copies (PSUM fp16 is not packed-viewable) all
regressed. Run-to-run noise is +-3 us with occasional +15-40 us
device-state excursions -- attribute changes only across repeated runs.
"""

import numpy as np
from contextlib import ExitStack

import concourse.bass as bass
import concourse.tile as tile
from concourse import mybir
from concourse.masks import make_identity
from concourse.bass_utils import run_bass_kernel_spmd
import concourse.bass2jax as _b2j


def _split_multiwait(bir: dict) -> int:
    """Walrus in this container rejects >1 sync-wait per instruction.

    Hoist excess waits onto NoOps inserted just before the instruction on
    the same engine (program order within the engine stream preserves the
    wait semantics exactly).
    """
    n = 0
    for fn in bir["functions"]:
        for blk in fn["blocks"]:
            out = []
            for inst in blk["instructions"]:
                si = inst.get("sync_info")
                waits = si.get("on_wait") if si else None
                if waits and len(waits) > 1:
                    for w in waits[:-1]:
                        out.append(
                            {
                                "opcode": "NoOp",
                                "engine": inst["engine"],
                                "name": f"{inst['name']}-xw{n}",
                                "ins": [],
                                "outs": [],
                                "sync_info": {"on_update": [], "on_wait": [w]},
                            }
                        )
                        n += 1
                    si["on_wait"] = [waits[-1]]
                out.append(inst)
            blk["instructions"] = out
    return n


_orig_compile_bir_kernel = _b2j.compile_bir_kernel


def _legalizing_compile_bir_kernel(ant_bir_str, *args, **kwargs):
    import orjson

    bir = orjson.loads(ant_bir_str)
    _split_multiwait(bir)
    return _orig_compile_bir_kernel(orjson.dumps(bir), *args, **kwargs)


_b2j.compile_bir_kernel = _legalizing_compile_bir_kernel

F32 = mybir.dt.float32
F16 = mybir.dt.float16
Alu = mybir.AluOpType
Act = mybir.ActivationFunctionType
Axis = mybir.AxisListType

B, N, D = 8192, 64, 128
NCORES = 8
BPC = B // NCORES          # 1024 batches per core
GROUP = 128                # batches per group
NGROUPS = BPC // GROUP     # 8
NTILES = GROUP // 2        # 64 two-batch tiles per group
NPAIR = GROUP // 2         # 64 batch-pairs per group (softmax partitions)
NH = GROUP // 128          # 1 c-half per group
CH = 8                     # tiles per transpose/copy chunk
NCH = NTILES // CH         # 8 chunks per group
LAG = 3                    # dot-chunk k trails transpose-chunk k+LAG


class _St:
    """Per-group pipeline state carried between build phases."""

    def __init__(self, g):
        self.g = g
        self.p16 = None
        self.tw = None
        self.w = None
        self.nmx = None
        self.wn16 = None
        self.wmat16 = None


class _Ctx:
    def __init__(self, tc, pools, consts, aps):
        self.tc = tc
        self.nc = tc.nc
        (self.p_p16, self.p_pt, self.p_small, self.ps_pt, self.ps_mm,
         self.ps_small) = pools
        self.ident16, self.ident32 = consts
        (self.pref_rows, self.u_all, self.ct16a, self.tpa, self.tca,
         self.ntca, self.cg16, self.wmat_ring) = aps


def _phase_a(cx, g, p16):
    """tw + interleaved transpose/dot passes + dots extraction + add/max."""
    nc = cx.nc
    st = _St(g)
    st.p16 = p16

    # pref chunk DMAs for this group (group 0's first chunk was emitted
    # ahead of the identity build in _build_nc); last group uses smaller
    # chunks so the drain after the final chunk is short
    r0 = g * GROUP * N
    HT = 16 if g < NGROUPS - 1 else 8
    for h0 in range(16 if g == 0 else 0, NTILES, HT):
        rh = r0 + h0 * 128
        nc.gpsimd.dma_start(
            out=p16[:, h0 : h0 + HT, :],
            in_=cx.pref_rows[rh : rh + HT * 128, :].rearrange(
                "(t p) d -> p t d", p=128
            ),
        )

    st.tw = cx.p_small.tile([NPAIR, 2, N], F32, tag="tw", name=f"tw{g}")
    for s in range(2):
        nc.scalar.activation(
            out=st.tw[:, s, :],
            in_=cx.tpa[:, g, s, :],
            func=Act.Abs,
            bias=cx.ntca[:, g, s : s + 1],
            scale=1.0,
        )
    nc.vector.reciprocal(out=st.tw[:], in_=st.tw[:])

    pts = cx.p_pt.tile([128, NTILES, 128], F16, tag="pts", name=f"pts{g}")
    ps_dots = cx.ps_mm.tile(
        [128, NTILES, 2], F32, tag="mm_ps", name=f"dots{g}"
    )

    def t_chunk(k):
        t0 = k * CH
        pt_ps = cx.ps_pt.tile(
            [128, CH, 128], F16, tag="pt_ps", name=f"ptps{g}_{k}"
        )
        for i in range(CH):
            nc.tensor.transpose(
                out=pt_ps[:, i, :],
                in_=p16[:, t0 + i, :],
                identity=cx.ident16[:],
            )
        if k < 5:
            nc.scalar.copy(out=pts[:, t0 : t0 + CH, :], in_=pt_ps[:])
        else:
            nc.vector.tensor_copy(out=pts[:, t0 : t0 + CH, :], in_=pt_ps[:])

    def d_chunk(k):
        t0 = k * CH
        for i in range(CH):
            t = t0 + i
            th, tr = divmod(t, 128 // 2)
            nc.tensor.matmul(
                out=ps_dots[:, t, :],
                lhsT=pts[:, t, :],
                rhs=cx.ct16a[:, NH * g + th, 2 * tr : 2 * tr + 2],
                start=(i == 0),
                stop=(i == CH - 1),
            )

    for k in range(NCH):
        t_chunk(k)
        if k >= LAG:
            d_chunk(k - LAG)
    for k in range(NCH - LAG, NCH):
        d_chunk(k)

    # valid dots sit at [row, parity=row//64]: extract the two halves
    dotw = cx.p_small.tile([128, NTILES], F32, tag="dotw", name=f"dotw{g}")
    nc.scalar.copy(out=dotw[0:64, :], in_=ps_dots[0:64, :, 0])
    nc.scalar.copy(out=dotw[64:128, :], in_=ps_dots[64:128, :, 1])

    # transpose [128(row), nt] -> [nt, 128(row)] => pair-major dots
    dr_ps = cx.ps_small.tile([NPAIR, 128], F32, tag="sm_ps", name=f"dr{g}")
    nc.tensor.transpose(out=dr_ps[:], in_=dotw[:], identity=cx.ident32[:])

    st.w = cx.p_small.tile([NPAIR, 2, N], F32, tag="w", name=f"w{g}")
    nc.vector.tensor_add(
        out=st.w[:],
        in0=dr_ps[:].rearrange("t (two n) -> t two n", two=2),
        in1=st.tw[:],
    )
    st.nmx = cx.p_small.tile([NPAIR, 2], F32, tag="nmx", name=f"nmx{g}")
    nc.vector.tensor_reduce(
        out=st.nmx[:], in_=st.w[:], axis=Axis.X, op=Alu.max, negate=True
    )
    _phase_b1(cx, st)
    return st


def _phase_b1(cx, st):
    """Softmax tail: exp + sum + reciprocal + normalize (no PE)."""
    nc = cx.nc
    g = st.g
    e = cx.p_small.tile([NPAIR, 2, N], F32, tag="e", name=f"e{g}")
    for s in range(2):
        nc.scalar.activation(
            out=e[:, s, :],
            in_=st.w[:, s, :],
            func=Act.Exp,
            bias=st.nmx[:, s : s + 1],
            scale=1.0,
        )
    z = cx.p_small.tile([NPAIR, 2], F32, tag="z", name=f"z{g}")
    nc.vector.reduce_sum(out=z[:], in_=e[:], axis=Axis.X)
    rz = cx.p_small.tile([NPAIR, 2], F32, tag="rz", name=f"rz{g}")
    nc.vector.reciprocal(out=rz[:], in_=z[:])
    st.wn16 = cx.p_small.tile([NPAIR, 2, N], F16, tag="wn16", name=f"wn{g}")
    for s in range(2):
        nc.scalar.activation(
            out=st.wn16[:, s, :],
            in_=e[:, s, :],
            func=Act.Copy,
            scale=rz[:, s : s + 1],
        )


def _phase_b2(cx, st):
    """W_MAT build: PE transpose of wn16 + block scatter."""
    nc = cx.nc
    g = st.g
    wc_ps = cx.ps_small.tile([128, NTILES], F16, tag="sm_ps", name=f"wc{g}")
    nc.tensor.transpose(
        out=wc_ps[:],
        in_=st.wn16[:].rearrange("t two n -> t (two n)"),
        identity=cx.ident16[0:NPAIR, 0:NPAIR],
    )
    wcol = cx.p_small.tile([128, NTILES], F16, tag="wcol", name=f"wcol{g}")
    nc.vector.tensor_copy(out=wcol[:], in_=wc_ps[:])
    # persistent pre-zeroed ring: only the data halves are ever written,
    # the zero halves survive across generations
    st.wmat16 = cx.wmat_ring[g % len(cx.wmat_ring)]
    nc.vector.tensor_copy(out=st.wmat16[0:64, :, 0], in_=wcol[0:64, :])
    nc.vector.tensor_copy(out=st.wmat16[64:128, :, 1], in_=wcol[64:128, :])


def _phase_c(cx, st):
    """Weighted-sum matmuls + u extraction (ACT) + store (sync)."""
    nc = cx.nc
    g = st.g
    b0 = g * GROUP
    HB = NTILES // 2
    for h in range(2):
        ps_ut = cx.ps_mm.tile(
            [128, HB, 2], F32, tag="mm_ps", name=f"ut{g}_{h}"
        )
        for k in range(HB):
            t = h * HB + k
            nc.tensor.matmul(
                out=ps_ut[:, k, :],
                lhsT=st.p16[:, t, :],
                rhs=st.wmat16[:, t, :],
                start=(k == 0),
                stop=(k == HB - 1),
            )
        uts = cx.p_small.tile(
            [128, GROUP // 2], F32, tag="uts", name=f"uts{g}_{h}"
        )
        nc.vector.tensor_copy(
            out=uts[:], in_=ps_ut[:].rearrange("d t two -> d (t two)")
        )
        bh = b0 + h * (GROUP // 2)
        nc.sync.dma_start(
            out=cx.u_all[:, bh : bh + GROUP // 2], in_=uts[:]
        )


def _emit_ct(cx, g):
    """PE transposes of group g's c halves into ct16a (+ scalar copies)."""
    nc = cx.nc
    for h in range(NH):
        gh = NH * g + h
        ct_ps = cx.ps_small.tile([128, 128], F16, tag="sm_ps", name=f"ct{gh}")
        nc.tensor.transpose(
            out=ct_ps[:],
            in_=cx.cg16[:, gh, :],
            identity=cx.ident16[:],
        )
        nc.vector.tensor_copy(out=cx.ct16a[:, gh, :], in_=ct_ps[:])


def _build_nc():
    nc = bass.Bass()
    pref = nc.declare_dram_parameter("pref", [BPC, N, D], F32, isOutput=False)
    c = nc.declare_dram_parameter("c", [BPC, 1, D], F32, isOutput=False)
    t_pref = nc.declare_dram_parameter("t_pref", [BPC, 1, N], F32, isOutput=False)
    t_c = nc.declare_dram_parameter("t_c", [BPC, 1], F32, isOutput=False)
    # u stored transposed [D, BPC] (direct from the weighted-sum PSUM
    # layout -- skips two PE transposes + copies per group); the host
    # transposes it back.
    u = nc.declare_dram_parameter("u", [D, BPC], F32, isOutput=True)

    pref_rows = pref[:].rearrange("b n d -> (b n) d")
    c_all = c[:].rearrange("b one d -> (b one) d")
    tp_all = t_pref[:].rearrange("b one n -> (b one) n")
    tc_all = t_c[:]
    u_all = u[:]

    with ExitStack() as ctx:
        tc = ctx.enter_context(tile.TileContext(nc))
        p_const = ctx.enter_context(tc.tile_pool(name="const", bufs=1))
        p_pre = ctx.enter_context(tc.tile_pool(name="pre", bufs=1))
        p_p16 = ctx.enter_context(tc.tile_pool(name="p16", bufs=4))
        p_pt = ctx.enter_context(tc.tile_pool(name="pt", bufs=3))
        p_small = ctx.enter_context(tc.tile_pool(name="small", bufs=3))
        ps_pt = ctx.enter_context(tc.tile_pool(name="ps_pt", bufs=3, space="PSUM"))
        ps_mm = ctx.enter_context(tc.tile_pool(name="ps_mm", bufs=3, space="PSUM"))
        ps_small = ctx.enter_context(
            tc.tile_pool(name="ps_small", bufs=2, space="PSUM")
        )

        nb = NGROUPS * GROUP

        # sync stream head: group 0's c first (contiguous 128 KB -- it
        # gates group 0's dot matmuls), then t tensors, then the rest of c.
        NGH = NGROUPS * NH
        c32a = p_pre.tile([128, NGH, D], F32)
        nc.sync.dma_start(
            out=c32a[:, 0:NH, :],
            in_=c_all[0:GROUP, :].rearrange("(h b) d -> b h d", b=128),
        )
        tpa = p_pre.tile([NPAIR, NGROUPS, 2, N], F32)
        nc.sync.dma_start(
            out=tpa[:],
            in_=tp_all[0:nb, :].rearrange(
                "(g t two) n -> t g two n", t=NPAIR, two=2
            ),
        )
        tca = p_pre.tile([NPAIR, NGROUPS, 2], F32)
        nc.sync.dma_start(
            out=tca[:],
            in_=tc_all[0:nb, :].rearrange(
                "(g t two) one -> t g (two one)", t=NPAIR, two=2
            ),
        )
        nc.sync.dma_start(
            out=c32a[:, NH:, :],
            in_=c_all[GROUP:nb, :].rearrange("(g b) d -> b g d", b=128),
        )

        ntca = p_pre.tile([NPAIR, NGROUPS, 2], F32)
        nc.vector.tensor_scalar_mul(out=ntca[:], in0=tca[:], scalar1=-1.0)

        # gpsimd stream: group 0's first pref chunk, then the identity
        # build (gpsimd-only affine_select) -- identities are ready right
        # when chunk 0's data lands.
        p16s = []
        for _gi in range(NGROUPS):
            p16_t = p_p16.tile(
                [128, NTILES, D], F16, tag="p16", name=f"p16_{_gi}"
            )
            p16s.append(p16_t)
        nc.gpsimd.dma_start(
            out=p16s[0][:, 0:16, :],
            in_=pref_rows[0 : 16 * 128, :].rearrange("(t p) d -> p t d", p=128),
        )
        ident16 = p_const.tile([128, 128], F16)
        ident32 = p_const.tile([128, 128], F32)
        make_identity(nc, ident16[:])
        make_identity(nc, ident32[:])
        consts = (ident16, ident32)

        # c cast: group 0 first, rest later (gates nothing early)
        cg16 = p_pre.tile([128, NGH, D], F16)
        nc.vector.tensor_copy(out=cg16[:, 0:NH, :], in_=c32a[:, 0:NH, :])
        nc.vector.tensor_copy(out=cg16[:, NH:, :], in_=c32a[:, NH:, :])
        ct16a = p_pre.tile([128, NGH, 128], F16)  # [D, group-half, batch]

        wmat_ring = []
        for _wi in range(3):
            wm = p_pre.tile([128, NTILES, 2], F16, name=f"wmatr{_wi}")
            nc.vector.memset(wm[:], 0.0)
            wmat_ring.append(wm)

        aps = (pref_rows, u_all, ct16a, tpa, tca, ntca, cg16, wmat_ring)
        cx = _Ctx(tc, (p_p16, p_pt, p_small, ps_pt, ps_mm, ps_small),
                  consts, aps)

        _emit_ct(cx, 0)

        # software pipeline, coarse blocks (fine interleaving costs ~10ns
        # of extra semaphore latency per matmul -- measured):
        #   iter g: [softmax-tail g-1] [A: loads+transposes+dots g]
        #           [W_MAT g-1] [weighted-sum + store g-1] [cT g+1]
        # Each deferred block's inputs are ready ~7 us before PE reaches
        # it, so no engine queue ever waits mid-chain.
        pend = None
        for g in range(NGROUPS):
            st = _phase_a(cx, g, p16s[g])
            if pend is not None:
                _phase_c(cx, pend)
            _phase_b2(cx, st)
            if g + 1 < NGROUPS:
                _emit_ct(cx, g + 1)
            pend = st

        _phase_c(cx, pend)

    return nc


_NC_CACHE = None
LAST_RESULT = None


def kernel(pref, c, t_pref, t_c):
    global _NC_CACHE, LAST_RESULT
    if _NC_CACHE is None:
        _NC_CACHE = _build_nc()
    nc = _NC_CACHE

    pref = np.ascontiguousarray(pref, dtype=np.float32)
    c = np.ascontiguousarray(c, dtype=np.float32)
    t_pref = np.ascontiguousarray(t_pref, dtype=np.float32)
    t_c = np.ascontiguousarray(t_c, dtype=np.float32)

    in_maps = []
    for i in range(NCORES):
        s = slice(i * BPC, (i + 1) * BPC)
        in_maps.append(
            {"pref": pref[s], "c": c[s], "t_pref": t_pref[s], "t_c": t_c[s]}
        )

    res = run_bass_kernel_spmd(nc, in_maps, list(range(NCORES)))
    LAST_RESULT = res
    return np.ascontiguousarray(
        np.concatenate([r["u"].T for r in res.results], axis=0)
    ).reshape(B, 1, D)

